# revision 10
# baseline (speedup 1.0000x reference)
"""MultiHeadEMA on 8 Trainium2 NeuronCores — v5.

Channel-sharded: embed_dim=1024 -> 8 x 128 channels (= SBUF partitions).
The FFT conv is an order-2 IIR  y_n[l] = q_n y_n[l-1] + x[l],
out = silu(c0 y0 + c1 y1 + w x), decimated by 4 for the DVE scan:
Y_n[j] = c_n y_n[4j] = q^4 Y_n[j-1] + u_n[j], u_n[j] = sum_k c_n q^k x[4j-k].
u and the phase reconstructions run as diagonal matmuls on the tensor engine
(PSUM accumulation, silu evacuates); phases r=1..3 use weights q_n^r on Y
plus sigma-weights on x phases.  Phase r=0 runs fully OFF the PE:
pre_0 = (Y0+Y1) + w*x0 via DVE add + scalar-engine scale + gpsimd add
(DVE on the last batch to shorten the drain).

v5 vs the 66.5us v1 baseline (measured):
  - r=0 off-PE: 172 -> 156 matmuls (PE busy was 41.4us, 100% of span).
  - batch-0 x DMA split in halves; u-tap diags built before all others;
    c-coefficient chain computed before q-powers -> first matmul ~2.5us
    earlier (was gated at 13.2us by the 1MB DMA + 18-deep diag queue).
  - sigmoid via 0.5*tanh(x/2)+0.5: tanh shares the silu act table, so the
    second ACT_TABLE_LOAD disappears.
  - last batch: per-phase output DMAs.
A full multi-engine rebalance (v4) was tried and measured SLOWER (77us):
AP-scalar tensor_scalar runs ~2.2cyc/elem, gpsimd lacks PSUM access and
runs adds at ~2.3us/KB, 512-chunked scans degrade to 3ns/elem, and the
power governor couples engine speeds — so the PE keeps the bulk.
"""

import numpy as np
import ml_dtypes

import concourse.bass as bass
import concourse.bacc as bacc
import concourse.tile as tile
from concourse import mybir
from concourse.bass_utils import run_bass_kernel_spmd

SEQ_LEN, BSZ, EMBED_DIM, NDIM = 4096, 4, 1024, 2
N_CORES = 8
D_PER = EMBED_DIM // N_CORES  # 128 channels/core = full SBUF partitions
SCALE = (1.0 / NDIM) ** 0.5
DEC = 4                   # decimation factor
J = SEQ_LEN // DEC        # decimated length 1024
CH = 512                  # matmul chunk (one fp32 PSUM bank)
NG = J // CH              # j-groups per slab (2)
F32 = mybir.dt.float32
BF16 = mybir.dt.bfloat16
AF = mybir.ActivationFunctionType
ALU = mybir.AluOpType
NBLK = 4


def build_bass():
    nc = bacc.Bacc(name="multihead_ema")
    x = nc.dram_tensor("x", [D_PER, BSZ, NBLK, J], BF16, kind="ExternalInput")
    # coef columns: [delta0, delta1, alpha0, alpha1, beta0, beta1, gamma0, gamma1, omega]
    coef = nc.dram_tensor("coef", [D_PER, 9], F32, kind="ExternalInput")
    eye = nc.dram_tensor("eye", [D_PER, D_PER], BF16, kind="ExternalInput")
    out = nc.dram_tensor("out", [D_PER, BSZ, DEC, J], BF16, kind="ExternalOutput")

    with tile.TileContext(nc) as tc:
        with (
            tc.tile_pool(name="const", bufs=1) as const,
            tc.tile_pool(name="xup", bufs=4) as xup,
            tc.tile_pool(name="yp", bufs=3) as yp,
            tc.tile_pool(name="tmp", bufs=4) as tmp,
            tc.tile_pool(name="op", bufs=4) as op,
            tc.tile_pool(name="psu", bufs=2, space="PSUM") as psu,
            tc.tile_pool(name="psc", bufs=2, space="PSUM") as psc,
        ):
            csb = const.tile([D_PER, 9], F32)
            nc.sync.dma_start(out=csb[:, :], in_=coef[:, :])
            eyesb = const.tile([D_PER, D_PER], BF16)
            nc.sync.dma_start(out=eyesb[:, :], in_=eye[:, :])

            # prefetch x; batch 0 in halves so the first u-matmuls start on
            # a 512KB transfer instead of 1MB
            xus = []
            for b in range(BSZ):
                xu = xup.tile([D_PER, 4, J], BF16, tag="xu")
                if b == 0:
                    nc.sync.dma_start(out=xu[:, :, 0:CH], in_=x[:, b, :, 0:CH])
                    nc.sync.dma_start(out=xu[:, :, CH:J], in_=x[:, b, :, CH:J])
                else:
                    nc.sync.dma_start(out=xu[:, :, :], in_=x[:, b, :, :])
                xus.append(xu)

            # --- coefficients. sigmoid(v) = 0.5*tanh(v/2)+0.5 (tanh shares
            # the silu table).  c-chain first: it gates the first matmul.
            th = const.tile([D_PER, 4], F32)
            nc.scalar.activation(out=th[:, :], in_=csb[:, 0:4], func=AF.Tanh,
                                 scale=0.5)
            sig = const.tile([D_PER, 4], F32)  # [p0, p1, sa0, sa1]
            nc.vector.tensor_scalar(out=sig[:, :], in0=th[:, :],
                                    scalar1=0.5, scalar2=0.5,
                                    op0=ALU.mult, op1=ALU.add)
            c1t = const.tile([D_PER, NDIM], F32)
            nc.vector.tensor_mul(out=c1t[:, :], in0=sig[:, 0:2], in1=csb[:, 4:6])
            cc = const.tile([D_PER, NDIM], F32)  # c_n = p beta gamma scale
            nc.vector.tensor_mul(out=cc[:, :], in0=c1t[:, :], in1=csb[:, 6:8])
            nc.vector.tensor_scalar_mul(out=cc[:, :], in0=cc[:, :],
                                        scalar1=SCALE)
            pq = const.tile([D_PER, NDIM], F32)
            nc.vector.tensor_mul(out=pq[:, :], in0=sig[:, 0:2], in1=sig[:, 2:4])
            q = const.tile([D_PER, NDIM], F32)  # q = 1 - p*sigmoid(alpha)
            nc.vector.tensor_scalar(out=q[:, :], in0=pq[:, :],
                                    scalar1=-1.0, scalar2=1.0,
                                    op0=ALU.mult, op1=ALU.add)
            _dn = [0]

            def diag(scalar_ap):
                _dn[0] += 1
                t = const.tile([D_PER, D_PER], BF16, tag=f"diag{_dn[0]}")
                nc.vector.tensor_scalar_mul(out=t[:, :], in0=eyesb[:, :],
                                            scalar1=scalar_ap)
                return t

            # u-tap weights, each built as soon as its coefficient exists
            # (they gate the first matmuls)
            w_cy = [[diag(cc[:, n:n + 1]) for n in range(NDIM)]]
            cq = const.tile([D_PER, NDIM], F32)   # c q
            nc.vector.tensor_mul(out=cq[:, :], in0=cc[:, :], in1=q[:, :])
            w_cy.append([diag(cq[:, n:n + 1]) for n in range(NDIM)])
            cq2 = const.tile([D_PER, NDIM], F32)  # c q^2
            nc.vector.tensor_mul(out=cq2[:, :], in0=cq[:, :], in1=q[:, :])
            w_cy.append([diag(cq2[:, n:n + 1]) for n in range(NDIM)])
            cq3 = const.tile([D_PER, NDIM], F32)  # c q^3
            nc.vector.tensor_mul(out=cq3[:, :], in0=cq2[:, :], in1=q[:, :])
            w_cy.append([diag(cq3[:, n:n + 1]) for n in range(NDIM)])

            q2 = const.tile([D_PER, NDIM], F32)
            nc.vector.tensor_mul(out=q2[:, :], in0=q[:, :], in1=q[:, :])
            q3 = const.tile([D_PER, NDIM], F32)
            nc.vector.tensor_mul(out=q3[:, :], in0=q2[:, :], in1=q[:, :])
            q4 = const.tile([D_PER, NDIM], F32)
            nc.vector.tensor_mul(out=q4[:, :], in0=q2[:, :], in1=q2[:, :])

            # pair-1 (phases 2,3) weights next, then pair-0's
            w_q2 = [diag(q2[:, n:n + 1]) for n in range(NDIM)]
            w_q3 = [diag(q3[:, n:n + 1]) for n in range(NDIM)]
            sig0 = const.tile([D_PER, 1], F32)    # c0 + c1 + w
            nc.vector.tensor_add(out=sig0[:, :], in0=cc[:, 0:1], in1=cc[:, 1:2])
            nc.vector.tensor_add(out=sig0[:, :], in0=sig0[:, :], in1=csb[:, 8:9])
            s1 = const.tile([D_PER, 1], F32)      # c0 q0 + c1 q1
            nc.vector.tensor_add(out=s1[:, :], in0=cq[:, 0:1], in1=cq[:, 1:2])
            s2 = const.tile([D_PER, 1], F32)      # c0 q0^2 + c1 q1^2
            nc.vector.tensor_add(out=s2[:, :], in0=cq2[:, 0:1], in1=cq2[:, 1:2])
            w_cw = diag(sig0[:, 0:1])
            w_cqs = diag(s1[:, 0:1])
            w_cq2s = diag(s2[:, 0:1])
            w_q1 = [diag(q[:, n:n + 1]) for n in range(NDIM)]

            q4b = [q4[:, n:n + 1].to_broadcast([D_PER, J]) for n in range(NDIM)]

            for b in range(BSZ):
                xu = xus[b]
                last = b == BSZ - 1

                # --- u_n in PSUM, Y_n = scan(q_n^4, u_n).  Batch 0 runs
                # k-outer across both EMA dims so the four k=0 matmuls (the
                # only weight ready early) cover the diag-build chain.
                def u_mm(pu, n, k, g):
                    s = bass.ts(g, CH)
                    if k == 0:
                        nc.tensor.matmul(pu[:, s], w_cy[0][n][:, :],
                                         xu[:, 0, s], start=True, stop=False)
                    elif g == 0:
                        nc.tensor.matmul(pu[:, 1:CH], w_cy[k][n][:, :],
                                         xu[:, 4 - k, 0:CH - 1],
                                         start=False, stop=(k == 3))
                    else:
                        nc.tensor.matmul(
                            pu[:, s], w_cy[k][n][:, :],
                            xu[:, 4 - k, g * CH - 1:(g + 1) * CH - 1],
                            start=False, stop=(k == 3))

                Y = []
                if b == 0:
                    pus = [psu.tile([D_PER, J], F32, tag="u", name=f"pu{n}")
                           for n in range(NDIM)]
                    for k in range(4):
                        for n in range(NDIM):
                            for g in range(NG):
                                u_mm(pus[n], n, k, g)
                    for n in range(NDIM):
                        yn = yp.tile([D_PER, J], BF16, tag=f"y{n}")
                        nc.vector.tensor_tensor_scan(
                            out=yn[:, :], data0=q4b[n], data1=pus[n][:, :],
                            initial=0.0, op0=ALU.mult, op1=ALU.add,
                        )
                        Y.append(yn)
                else:
                    for n in range(NDIM):
                        pu = psu.tile([D_PER, J], F32, tag="u")
                        for k in range(4):
                            for g in range(NG):
                                u_mm(pu, n, k, g)
                        yn = yp.tile([D_PER, J], BF16, tag=f"y{n}")
                        nc.vector.tensor_tensor_scan(
                            out=yn[:, :], data0=q4b[n], data1=pu[:, :],
                            initial=0.0, op0=ALU.mult, op1=ALU.add,
                        )
                        Y.append(yn)

                # --- r=0 fully off the PE: pre0 = (Y0+Y1) + w*x0
                t0 = tmp.tile([D_PER, J], BF16, tag="t0", bufs=2)
                nc.scalar.activation(out=t0[:, :], in_=xu[:, 0, :],
                                     func=AF.Copy, scale=csb[:, 8:9])
                ysum = tmp.tile([D_PER, J], BF16, tag="ysum", bufs=2)
                nc.vector.tensor_add(out=ysum[:, :], in0=Y[0][:, :],
                                     in1=Y[1][:, :])
                pre0 = tmp.tile([D_PER, J], BF16, tag="pre0", bufs=2)
                eng0 = nc.vector if last else nc.gpsimd
                eng0.tensor_add(out=pre0[:, :], in0=ysum[:, :], in1=t0[:, :])

                # --- phases 2,3 then 1 in PSUM; silu evacuates
                ob = op.tile([D_PER, DEC, J], BF16, tag="ob", bufs=2)
                for g in range(NG):
                    s = bass.ts(g, CH)
                    pt = psc.tile([D_PER, 2 * CH], F32, tag="cmb")
                    for h in range(2):
                        r = 2 + h
                        tgt = pt[:, bass.ts(h, CH)]
                        wq = w_q2 if r == 2 else w_q3
                        if r == 2:
                            xw = [(w_cw, 2), (w_cqs, 1)]
                        else:
                            xw = [(w_cw, 3), (w_cqs, 2), (w_cq2s, 1)]
                        for i, (wt, rr) in enumerate(xw):
                            nc.tensor.matmul(tgt, wt[:, :], xu[:, rr, s],
                                             start=(i == 0), stop=False)
                        nc.tensor.matmul(tgt, wq[0][:, :], Y[0][:, s],
                                         start=False, stop=False)
                        nc.tensor.matmul(tgt, wq[1][:, :], Y[1][:, s],
                                         start=False, stop=True)
                    in_ap = pt[:, :].rearrange("p (h k) -> p h k", h=2)
                    nc.scalar.activation(out=ob[:, 2:4, s], in_=in_ap,
                                         func=AF.Silu)
                if not last:
                    nc.sync.dma_start(out=out[:, b, 2:4, :], in_=ob[:, 2:4, :])
                else:
                    nc.sync.dma_start(out=out[:, b, 2:3, :], in_=ob[:, 2:3, :])
                    nc.sync.dma_start(out=out[:, b, 3:4, :], in_=ob[:, 3:4, :])

                for g in range(NG):
                    s = bass.ts(g, CH)
                    pt = psc.tile([D_PER, 2 * CH], F32, tag="cmb")
                    tgt = pt[:, 0:CH]
                    nc.tensor.matmul(tgt, w_cw[:, :], xu[:, 1, s],
                                     start=True, stop=False)
                    nc.tensor.matmul(tgt, w_q1[0][:, :], Y[0][:, s],
                                     start=False, stop=False)
                    nc.tensor.matmul(tgt, w_q1[1][:, :], Y[1][:, s],
                                     start=False, stop=True)
                    nc.scalar.activation(out=ob[:, 1, s], in_=tgt,
                                         func=AF.Silu)
                nc.scalar.activation(out=ob[:, 0, :], in_=pre0[:, :],
                                     func=AF.Silu)
                if not last:
                    nc.sync.dma_start(out=out[:, b, 0:2, :], in_=ob[:, 0:2, :])
                else:
                    nc.sync.dma_start(out=out[:, b, 1:2, :], in_=ob[:, 1:2, :])
                    nc.sync.dma_start(out=out[:, b, 0:1, :], in_=ob[:, 0:1, :])

    nc.compile()
    return nc


_CACHE: dict = {}


def _get_nc():
    if "nc" not in _CACHE:
        _CACHE["nc"] = build_bass()
    return _CACHE["nc"]


def make_in_maps(inputs):
    x = np.asarray(inputs["x"], np.float32)
    delta = np.asarray(inputs["delta"], np.float32).reshape(EMBED_DIM, NDIM)
    alpha = np.asarray(inputs["alpha"], np.float32).reshape(EMBED_DIM, NDIM)
    beta = np.asarray(inputs["beta"], np.float32).reshape(EMBED_DIM, NDIM)
    gamma = np.asarray(inputs["gamma"], np.float32).reshape(EMBED_DIM, NDIM)
    omega = np.asarray(inputs["omega"], np.float32).reshape(EMBED_DIM, 1)
    coef_full = np.concatenate([delta, alpha, beta, gamma, omega], axis=1)
    eye = np.eye(D_PER, dtype=ml_dtypes.bfloat16)
    in_maps = []
    for c in range(N_CORES):
        sl = slice(c * D_PER, (c + 1) * D_PER)
        xc = x[:, :, sl].transpose(2, 1, 0).astype(ml_dtypes.bfloat16)  # [128,B,L]
        ph = xc.reshape(D_PER, BSZ, J, DEC).transpose(0, 1, 3, 2)  # [128,B,4,J]
        in_maps.append(
            {"x": np.ascontiguousarray(ph),
             "coef": np.ascontiguousarray(coef_full[sl]), "eye": eye}
        )
    return in_maps


def gather_out(results):
    out = np.empty((SEQ_LEN, BSZ, EMBED_DIM), np.float32)
    for c in range(N_CORES):
        # [128, B, 4, J] phase-major -> [l = 4j+r, b, d]
        arr = results[c]["out"].astype(np.float32)
        out[:, :, c * D_PER : (c + 1) * D_PER] = arr.transpose(3, 2, 1, 0).reshape(
            SEQ_LEN, BSZ, D_PER
        )
    return out


def _run(inputs, **kwargs):
    nc = _get_nc()
    in_maps = make_in_maps(inputs)
    res = run_bass_kernel_spmd(nc, in_maps, core_ids=list(range(N_CORES)), **kwargs)
    return gather_out(res.results), res


def kernel(**inputs) -> np.ndarray:
    out, _ = _run(inputs)
    return out
